# revision 61
# baseline (speedup 1.0000x reference)
"""Trainium2 Bass kernel for CausalSelfAttention (GQA, RoPE, prefill).

Tensor-parallel over the 8 query groups: core g owns query heads
[4g, 4g+4) and kv head g.  Each core computes a partial output
(full-shape, f16) that the host sums in f32.

Per-core pipeline (one NeuronCore, Tile-scheduled).  Attention's serial
softmax chains are interleaved INTO the big GEMM k-loops as pacing
"steps" so the PE array never idles behind exp/normalization:

  A: qkv(b0) = wqkvT.T @ xT, 6-wide PSUM k-major.
  B: qkv(b1) in two 3-wide PSUM passes per 512-token chunk; rope (DVE)
     and v-transposes (PE) ride pass 1, attention(b0) heads ride the
     remaining k-loops: scoresT = kT.T @ qT per causal kv-chunk (PSUM)
     -> exp on ACT (QK scale folded in) -> mask -> PV matmuls.
     Softmax denominators: DVE compacts the exp trapezoid to [128,T]
     (csum), one ones-vector PE matmul row-sums it, reciprocal on DVE,
     then a DRAM-bounce DMA broadcasts 1/s to 128 partitions (DMA can
     broadcast from DRAM; gpsimd partition ops don't compile here).
  C: out-proj of b0 token tiles; attention(b1) + rope(b1 q-heads)
     packed 2-steps-per-slot into the early m-tiles.  Same PSUM pool as
     B (proj reuses the qkv tag) so there is no pool-handoff stall.
  D: out-proj of b1 token tiles; f16 partials DMA'd per 512-col block.

Cost-model exec: 315.5us vs 304.6us pure-GEMM roofline (PE busy 308.0,
97.6% occupancy); baseline was 368.6us.
"""

import numpy as np

B, T, NE, NH, NQG, HS = 2, 1024, 4096, 32, 8, 128
QPK = NH // NQG          # 4 query heads per kv group
NT = B * T               # 2048 tokens
GW = (QPK + 2) * HS      # 768 qkv rows per group
GQ = QPK * HS            # 512 q cols per group
P = 128
NCORES = 8
KC = NE // P             # 32 contraction chunks for qkv proj
MC = GW // P             # 6 qkv feature chunks (q0..q3, k, v)
TC8 = T // P             # 8 token chunks per batch
SCALE = 1.0 / float(np.sqrt(HS))

_CACHE = {}


def _split_waits(nc, mybir, max_waits=1):
    """walrus in this container rejects >1 sync-wait per instruction;
    hoist extras onto single-wait NoOps just before (equivalent since
    semaphores are monotonic and a sequencer executes in order)."""
    for fn in nc.m.functions:
        for blk in fn.blocks:
            new_list, changed = [], False
            for inst in blk.instructions:
                si = getattr(inst, "sync_info", None)
                if si is not None and len(si.on_wait) > max_waits:
                    waits = list(si.on_wait)
                    for i, w in enumerate(waits[:-max_waits]):
                        nop = mybir.InstNoOp(
                            name=f"{inst.name}-wsplit-{i}", ins=[], outs=[],
                            engine=inst.engine)
                        nop.sync_info = mybir.SyncInfo(on_wait=[w], on_update=[])
                        new_list.append(nop)
                    inst.sync_info = mybir.SyncInfo(
                        on_wait=waits[-max_waits:], on_update=list(si.on_update))
                    changed = True
                new_list.append(inst)
            if changed:
                blk.instructions = new_list


def _build_nc():
    import concourse.bass as bass
    import concourse.mybir as mybir
    import concourse.tile as tile
    from contextlib import ExitStack

    f32 = mybir.dt.float32
    f16 = mybir.dt.float16

    nc = bass.Bass()
    xT_d = nc.dram_tensor("xT", [NE, NT], f16, kind="ExternalInput")
    wqkvT_d = nc.dram_tensor("wqkvT", [NE, GW], f16, kind="ExternalInput")
    wprojT_d = nc.dram_tensor("wprojT", [GQ, NE], f16, kind="ExternalInput")
    cc_d = nc.dram_tensor("cc", [P, NT], f16, kind="ExternalInput")
    ss_d = nc.dram_tensor("ss", [P, NT], f16, kind="ExternalInput")
    mask_d = nc.dram_tensor("maskT", [P, P], f16, kind="ExternalInput")
    ident16_d = nc.dram_tensor("ident16", [P, P], f16, kind="ExternalInput")
    ones16_d = nc.dram_tensor("ones16", [P, 1], f16, kind="ExternalInput")
    sden_d = nc.dram_tensor("sden", [B * QPK, T], f16)  # 1/s bounce rows
    out_d = nc.dram_tensor("out", [NT, NE], f16, kind="ExternalOutput")

    # column offset of kv-chunk c's block inside the expT tile
    offs, acc = [], 0
    for c in range(TC8):
        offs.append(acc)
        acc += (TC8 - c) * P

    with tile.TileContext(nc) as tc:
        sL = ExitStack()
        try:
            const = sL.enter_context(tc.tile_pool(name="const", bufs=1))
            cc = const.tile([P, NT], f16)
            ss = const.tile([P, NT], f16)
            maskT = const.tile([P, P], f16)
            ident16 = const.tile([P, P], f16)
            ones16 = const.tile([P, 1], f16)

            qk = sL.enter_context(tc.tile_pool(name="qk", bufs=1, side="right"))
            q16 = [qk.tile([P, QPK, T], f16, name=f"q16_{b}")
                   for b in range(B)]
            k16 = [qk.tile([P, T], f16, name=f"k16_{b}") for b in range(B)]
            vtm = [qk.tile([P, TC8, P], f16, name=f"vtm_{b}")
                   for b in range(B)]
            y16 = [qk.tile([P, QPK, T], f16, name=f"y16_{b}")
                   for b in range(B)]

            wq_pool = sL.enter_context(tc.tile_pool(name="wq", bufs=1))
            wq = wq_pool.tile([P, KC, GW], f16)
            wqr = wqkvT_d[:].rearrange("(ko p) m -> p ko m", p=P)
            wp_pool = sL.enter_context(tc.tile_pool(name="wp", bufs=1))
            wp = wp_pool.tile([P, QPK, NE], f16)
            wpr = wprojT_d[:].rearrange("(kc p) n -> p kc n", p=P)

            qkv_pool = sL.enter_context(tc.tile_pool(name="qkv", bufs=1))
            qkv = qkv_pool.tile([P, MC, NT], f16)

            expT_pool = sL.enter_context(
                tc.tile_pool(name="expT", bufs=2, side="right"))
            den_pool = sL.enter_context(
                tc.tile_pool(name="den", bufs=2, side="right"))
            rp = sL.enter_context(tc.tile_pool(name="rope", bufs=2))
            ob_pool = sL.enter_context(tc.tile_pool(name="ob", bufs=4))
            xs_pool = sL.enter_context(tc.tile_pool(name="xs", bufs=10))

            # ---------- emission helpers ----------
            def qkv_pass(ps_pool, xs_pool, n, ms, steps, tag, dma_wq=False):
                """One k-major accumulation pass: tokens [n*512,(n+1)*512),
                feature chunks `ms`.  steps[k]() interleaves extra work."""
                tok0 = n * 512
                psums = {m: ps_pool.tile([P, 512], f32, tag=tag,
                                         bufs=len(ms), name=f"q{n}_{m}")
                         for m in ms}
                for k in range(KC):
                    if dma_wq:
                        nc.sync.dma_start(wq[:, k, :], wqr[:, k, :])
                    xt = xs_pool.tile([P, 512], f16, tag="xt",
                                      name=f"xt{n}_{ms[0]}_{k}")
                    nc.sync.dma_start(
                        xt[:], xT_d[k * P:(k + 1) * P, tok0:tok0 + 512])
                    for m in ms:
                        nc.tensor.matmul(
                            psums[m][:], wq[:, k, m * P:(m + 1) * P],
                            xt[:], start=(k == 0), stop=(k == KC - 1))
                    if k < len(steps) and steps[k] is not None:
                        steps[k]()
                for m in reversed(ms):   # v/k copies first: unblocks
                    nc.any.tensor_copy(  # v-transposes and k-rope sooner
                        qkv[:, m, tok0:tok0 + 512], psums[m][:])
                for s in steps[KC:]:
                    if s is not None:
                        s()

            def rope_rot(b, hc):
                """Stage 1 of RoPE: partition-rotated copy via 2 DMAs."""
                h = HS // 2
                tok = slice(b * T, (b + 1) * T)
                src = qkv[:, hc, tok]
                rot = rp.tile([P, T], f16, tag="rot", name=f"rot{b}_{hc}")
                nc.sync.dma_start(rot[0:h, :], src[h:P, :])
                nc.sync.dma_start(rot[h:P, :], src[0:h, :])
                return rot

            def rope_mul(b, hc, rot):
                """Stage 2 of RoPE: dst = src*cos + rot*(+-sin) on DVE."""
                tok = slice(b * T, (b + 1) * T)
                src = qkv[:, hc, tok]
                t1 = rp.tile([P, T], f16, tag="t1", bufs=1,
                             name=f"t1_{b}_{hc}")
                t2 = rp.tile([P, T], f16, tag="t2", bufs=1,
                             name=f"t2_{b}_{hc}")
                nc.vector.tensor_mul(t1[:], src, cc[:, tok])
                nc.vector.tensor_mul(t2[:], rot[:], ss[:, tok])
                dst = q16[b][:, hc, :] if hc < QPK else k16[b][:]
                nc.vector.tensor_add(dst, t1[:], t2[:])

            def rope_steps(b, hc):
                """RoPE as two pacing steps (rot-DMA, then muls)."""
                box = {}

                def s1():
                    box["rot"] = rope_rot(b, hc)

                def s2():
                    rope_mul(b, hc, box.pop("rot"))
                return [s1, s2]

            def rope_one(b, hc):
                rope_mul(b, hc, rope_rot(b, hc))

            def vt_one(ps_pool, b, c, tag="sacc", bufs=3):
                vt_ps = ps_pool.tile([P, P], f16, tag=tag, bufs=bufs,
                                     name=f"vt{b}_{c}")
                nc.tensor.transpose(
                    vt_ps[:], qkv[:, QPK + 1, b * T + c * P:b * T + (c + 1) * P],
                    ident16[:])
                nc.any.tensor_copy(vtm[b][:, c, :], vt_ps[:])

            def attn_steps(ps_pool, b, hc, sacc_bufs=2):
                """Attention for (batch b, query head hc) as a list of
                closures; caller interleaves them with GEMM filler.  None
                entries are pacing pads (no emission)."""
                qT_i = q16[b][:, hc, :]
                expT = expT_pool.tile([P, acc], f16, tag="expT",
                                      name=f"expT{b}_{hc}")
                yps = ps_pool.tile([P, T], f32, tag="yps", bufs=1,
                                   name=f"yps{b}_{hc}")
                csum = den_pool.tile([P, T], f16, tag="csum",
                                     name=f"cs{b}_{hc}")
                s_sb = den_pool.tile([1, T], f16, tag="s_sb",
                                     name=f"ssb{b}_{hc}")
                rb = den_pool.tile([P, T], f16, tag="rb", name=f"rb{b}_{hc}")
                steps = []

                def csum_add(c):
                    # fold chunk c of expT into the kv-compacted csum
                    if c == 0:
                        nc.vector.tensor_copy(csum[:], expT[:, 0:T])
                    else:
                        w = (TC8 - c) * P
                        nc.vector.tensor_add(
                            csum[:, c * P:T], csum[:, c * P:T],
                            expT[:, offs[c]:offs[c] + w])

                def s_chunk(c):
                    def go():
                        kT_c = k16[b][:, c * P:(c + 1) * P]
                        spans = [(c * P, 512)] if c < 4 else []
                        spans += [(max(512, c * P), T)]
                        for (q0, q1) in spans:
                            sps = ps_pool.tile([P, 512], f32, tag="sacc",
                                               bufs=sacc_bufs,
                                               name=f"sps{b}_{hc}_{c}_{q0}")
                            w = q1 - q0
                            nc.tensor.matmul(sps[:, :w], kT_c, qT_i[:, q0:q1],
                                             start=True, stop=True)
                            eo = offs[c] + (q0 - c * P)
                            nc.scalar.activation(
                                expT[:, eo:eo + w], sps[:, :w],
                                mybir.ActivationFunctionType.Exp, scale=SCALE)
                        # zero the invalid (kv > q) half of the diagonal block
                        nc.vector.tensor_mul(
                            expT[:, offs[c]:offs[c] + P],
                            expT[:, offs[c]:offs[c] + P], maskT[:])
                        if c > 0:
                            csum_add(c - 1)
                    return go

                for c in range(TC8):
                    steps.append(s_chunk(c))

                def pv(s0, s1, cs, last_c, extra=None):
                    def go():
                        for c in cs:
                            q0 = max(s0, c * P)
                            sl = slice(offs[c] + (q0 - c * P),
                                       offs[c] + (s1 - c * P))
                            nc.tensor.matmul(
                                yps[:, q0:s1], vtm[b][:, c, :], expT[:, sl],
                                start=(c == 0), stop=(c == last_c))
                        if extra:
                            extra()
                    return go

                def rowsum():
                    # s[q] = ones.T @ csum (kv compacted to 128 lanes)
                    for s0 in (0, 512):
                        s_ps = ps_pool.tile([1, 512], f32, tag="sacc",
                                            bufs=sacc_bufs,
                                            name=f"sp{b}_{hc}_{s0}")
                        nc.tensor.matmul(s_ps[:], ones16[:],
                                         csum[:, s0:s0 + 512],
                                         start=True, stop=True)
                        nc.any.tensor_copy(s_sb[:, s0:s0 + 512], s_ps[:])

                # span (0,512): chunks 0..3 only; span (512,T): chunks 0..7
                steps.append(pv(0, 512, range(0, 4), 3,
                                extra=lambda: csum_add(TC8 - 1)))
                steps.append(pv(512, T, range(0, 4), 7, extra=rowsum))
                steps.append(pv(512, T, range(4, TC8), 7))

                srow = b * QPK + hc

                def den_finish():
                    # 1/s on the single-partition row, bounce out to DRAM
                    with nc.allow_low_precision(
                            reason="1/s in f16: rel err ~5e-4, tol 2e-2"):
                        nc.vector.reciprocal(s_sb[:], s_sb[:])
                    nc.sync.dma_start(sden_d[srow:srow + 1, :], s_sb[:])
                steps.append(den_finish)

                def den_bc():
                    # DMA partition-broadcast of the 1/s row to 128 lanes
                    nc.sync.dma_start(
                        rb[:], sden_d[srow:srow + 1, :].to_broadcast([P, T]))
                steps.append(den_bc)

                def ynorm():
                    nc.vector.tensor_mul(y16[b][:, hc, :], yps[:], rb[:])
                steps.append(None)
                steps.append(ynorm)
                return steps

            def proj_mtile(ps_pool, m, steps):
                """Out-proj for token tile m (128 tokens); steps interleave
                after each 512-col block."""
                for n in range(NE // 512):
                    opsum = ps_pool.tile([P, 512], f32, tag="qB", bufs=3,
                                         name=f"ops{m}_{n}")
                    for kc in range(QPK):
                        nc.tensor.matmul(
                            opsum[:], y16[m // TC8][:, kc,
                                                    (m % TC8) * P:
                                                    (m % TC8 + 1) * P],
                            wp[:, kc, n * 512:(n + 1) * 512],
                            start=(kc == 0), stop=(kc == QPK - 1))
                    ob = ob_pool.tile([P, 512], f16, tag="ob",
                                      name=f"ob{m}_{n}")
                    nc.any.tensor_copy(ob[:], opsum[:])
                    nc.sync.dma_start(
                        out_d[m * P:(m + 1) * P, n * 512:(n + 1) * 512],
                        ob[:])
                    if n < len(steps) and steps[n] is not None:
                        steps[n]()
                for s in steps[NE // 512:]:
                    if s is not None:
                        s()

            # ---------- phase A: qkv(b0), 6-wide k-major ----
            with ExitStack() as sA:
                psA = sA.enter_context(
                    tc.tile_pool(name="psA", bufs=1, space="PSUM"))
                qkv_pass(psA, xs_pool, 0, list(range(MC)), [], "qA",
                         dma_wq=True)
                nc.sync.dma_start(cc[:], cc_d[:])
                nc.sync.dma_start(ss[:], ss_d[:])
                nc.sync.dma_start(maskT[:], mask_d[:])
                nc.sync.dma_start(ident16[:], ident16_d[:])
                nc.sync.dma_start(ones16[:], ones16_d[:])
                qkv_pass(psA, xs_pool, 1, list(range(MC)), [], "qA")

            # ---------- phase B: qkv(b1) two 3-wide passes; rope(b0) and
            # v-transposes ride pass 1 (PE-light, lets DVE catch up), then
            # attn(b0) heads ride the remaining k-loops ----------
            MG0 = [0, QPK, QPK + 1]   # q0, k, v  -> ready after 2nd pass
            MG1 = [1, 2, 3]           # q1..q3
            with ExitStack() as sB:
                psB = sB.enter_context(
                    tc.tile_pool(name="psB", bufs=1, space="PSUM"))
                xsB = xs_pool
                # k-rope first (attn b0 h0 needs it next pass), q0 second.
                # rot-DMA steps sit 8 k-iters apart so they never starve the
                # serial SP dispatcher of xt streams; pads before the
                # v-transposes let the deferred qkv copies land first.
                rk = rope_steps(0, QPK)
                r0 = rope_steps(0, 0)
                pre = ([None] * 4 + [rk[0], rk[1]] + [None] * 2 +
                       [r0[0], r0[1]] + [None] * 2)
                pre += [(lambda c=c: vt_one(psB, 0, c)) for c in range(TC8)]
                qkv_pass(psB, xsB, 2, MG0, pre, "qB")
                nc.sync.dma_start(wp[:, 0, :], wpr[:, 0, :])
                s30 = attn_steps(psB, 0, 0, sacc_bufs=3)
                for hc in (1, 2, 3):
                    s30 += rope_steps(0, hc)
                qkv_pass(psB, xsB, 3, MG0, s30, "qB")
                nc.sync.dma_start(wp[:, 1, :], wpr[:, 1, :])
                # mg0 of both n done: k/v/q0 of b1 usable
                h2 = attn_steps(psB, 0, 2, sacc_bufs=3)
                s21 = attn_steps(psB, 0, 1, sacc_bufs=3)
                s21 += rope_steps(1, QPK) + rope_steps(1, 0)
                s21 += [(lambda c=c: vt_one(psB, 1, c)) for c in range(TC8)]
                s21 += h2[:5]   # h2 head-start: h3's chain must clear by C
                qkv_pass(psB, xsB, 2, MG1, s21, "qB")
                nc.sync.dma_start(wp[:, 2, :], wpr[:, 2, :])
                s31 = h2[5:] + attn_steps(psB, 0, 3, sacc_bufs=3)
                qkv_pass(psB, xsB, 3, MG1, s31, "qB")
                nc.sync.dma_start(wp[:, 3, :], wpr[:, 3, :])

                # ---- phase C: proj(b0 tiles) + attn(b1); D: proj(b1) ----
                # same PSUM pool: proj accumulators reuse the qB tag slots,
                # so there is no pool handoff stall at the B/C boundary.
                psC = psB
                qsteps = []
                for hc in range(QPK):
                    h = attn_steps(psC, 1, hc, sacc_bufs=3)
                    # rope(b1, hc+1) rides just behind head hc's start: done
                    # a full head-window before head hc+1 needs it
                    ins = rope_steps(1, hc + 1) if hc + 1 < QPK else []
                    qsteps += h[:2] + ins + h[2:]
                # front-load: 2 steps/slot for m0-m3, then 1/slot, so the
                # last head's denominator chain clears well before proj(b1)
                counts = [16, 16, 16, 16, 2] + [0] * 11
                pos = 0
                for m in range(NT // P):
                    chunk = qsteps[pos:pos + counts[m]]
                    pos += counts[m]
                    if len(chunk) > 8:
                        it = iter(chunk)
                        chunk = [
                            (lambda a=a, b=b: ((a() if a else None),
                                               (b() if b else None)))
                            for a, b in zip(it, it)]
                    proj_mtile(psC, m, chunk)
        finally:
            sL.close()

    _split_waits(nc, mybir)
    return nc


def _host_prep(x, cos, sin, W_attn, W_proj):
    xT = np.ascontiguousarray(x.reshape(NT, NE).T.astype(np.float16))
    cosT = np.tile(cos.T, (1, B))
    sinT = np.tile(sin.T, (1, B))
    cc = np.ascontiguousarray(
        np.concatenate([cosT, cosT], axis=0)).astype(np.float16)
    ss = np.ascontiguousarray(
        np.concatenate([-sinT, sinT], axis=0)).astype(np.float16)
    # scoresT layout [kv, q]: zero strictly-lower (kv > q) entries post-exp
    maskT = np.triu(np.ones((P, P), dtype=np.float16))
    common = {"xT": xT, "cc": cc, "ss": ss, "maskT": maskT,
              "ident16": np.eye(P, dtype=np.float16),
              "ones16": np.ones((P, 1), dtype=np.float16)}
    in_maps = []
    for g in range(NCORES):
        m = dict(common)
        m["wqkvT"] = np.ascontiguousarray(
            W_attn[g * GW:(g + 1) * GW, :].T.astype(np.float16))
        m["wprojT"] = np.ascontiguousarray(
            W_proj[:, g * GQ:(g + 1) * GQ].T.astype(np.float16))
        in_maps.append(m)
    return in_maps


LAST_EXEC_NS = None
LAST_RES = None


def kernel(x, cos, sin, W_attn, W_proj, max_seq_length):
    global LAST_EXEC_NS, LAST_RES
    from concourse.bass_utils import run_bass_kernel_spmd

    x = np.asarray(x, dtype=np.float32)
    cos = np.asarray(cos, dtype=np.float32)
    sin = np.asarray(sin, dtype=np.float32)
    W_attn = np.asarray(W_attn, dtype=np.float32)
    W_proj = np.asarray(W_proj, dtype=np.float32)

    if "nc" not in _CACHE:
        _CACHE["nc"] = _build_nc()
    nc = _CACHE["nc"]

    in_maps = _host_prep(x, cos, sin, W_attn, W_proj)
    res = run_bass_kernel_spmd(nc, in_maps, core_ids=list(range(NCORES)))
    LAST_EXEC_NS = res.exec_time_ns
    LAST_RES = res

    acc = res.results[0]["out"].astype(np.float32)
    for g in range(1, NCORES):
        acc = acc + res.results[g]["out"].astype(np.float32)
    return acc.reshape(B, T, NE)
